# revision 17
# baseline (speedup 1.0000x reference)
"""BOW classifier kernel for 8 Trainium2 NeuronCores.

Data-parallel over the batch dim (128 columns per core).  The embedding
mean-pool is a dense count matmul: A[v, b] = count of token v in column
b's valid prefix, restricted to the rows the core references; the device
computes pooled*len = A^T @ emb_used on the tensor engine.

v2: the dominant embedding stream ships in fp8 (e4m3, table pre-scaled by
8) instead of fp16, halving HBM bytes.  Quantization noise is averaged
down by the mean-pool for long columns, but short columns (len <= 64)
get no averaging, so their tokens are routed through a small exact-fp16
side stream instead (also any token with a count > 15, which fp8 cannot
represent).  Counts ship directly as fp8/fp16 *interleaved with the
embedding rows in one stream tensor* ([count row | emb row] per chunk),
so every chunk arrives in a single DMA line and there is no on-device
unpack work at all -- the vector engine only touches the tiny MLP tail.
The fp8 matmuls run in DoubleRow perf mode (two 128-row chunks per
instruction, 2x PE throughput) so the tensor engine outruns the DMA
stream even while HAM-throttled cold.  Stream tiles ramp up (PE starts
after a ~0.2 MB tile) and taper down (short tail after the last DMA).
The MLP tail runs transposed (hT = W1^T @ pooled^T) in fp16 with biases
folded in as ones-row matmuls, and the [128, 2] result is transposed so
the store is two DMA records.
"""

import sys

import numpy as np
import ml_dtypes

for _p in ("/opt/trn_rl_repo",):
    if _p not in sys.path:
        sys.path.insert(0, _p)

V, E, H, O = 50000, 300, 512, 2
S, B = 512, 1024
NCORES = 8
BS = B // NCORES   # 128 batch columns per core
SCALE = 8.0        # table pre-scale folded into the length divisor
TSHORT = 64        # columns with len <= TSHORT take the exact fp16 path
EP = 304           # emb bytes per fp8 row (300 + pad, 16B aligned)
CW8 = 128 + EP     # fp8 stream: 128 count bytes + emb row per chunk
CW16 = 128 + EP    # fp16 stream: 128 count elems + emb row (in fp16 elems)
G8 = 18            # chunks per fp8 DMA tile (finer tiles smooth the chase)
G16 = 18           # chunks per fp16 DMA tile (18*864 B line < 16 KB)
NBUF = 10          # rotating stream buffers
NWARM = 12         # fp32 warm-up matmuls: keep the PE busy while the first
                   # stream tile is in flight so HAM reaches 2.4 GHz by then
# aux-tensor layout (fp16 cols): 3x512 w1 chunks, 4 w2 chunks at 64-B slots,
# b2, lengths*SCALE -- one DMA instead of ~1500 tiny records
AUX_W2 = [1536 + 32 * j for j in range(4)]
AUX_B2 = 1664
AUX_LEN = 1696
AUXW = 1728

F8 = ml_dtypes.float8_e4m3  # TRN FP8_EXP4 (IEEE e4m3, max 240)


def _plan_long(n):
    """Tile sizes for the fp8 stream: small head tiles so the PE starts
    early, full tiles in the middle, small tail so the PE catch-up after
    the last DMA is short."""
    tiles = []
    for g in (2, 4, 8):
        if n <= 0:
            break
        g = min(g, n)
        tiles.append(g)
        n -= g
    while n > G8:
        tiles.append(G8)
        n -= G8
    if n > 8:
        tiles += [n - 4, 4]
    elif n:
        tiles.append(n)
    return tiles


def _plan_chunks(n, gmax):
    tiles = []
    while n > 0:
        g = min(gmax, n)
        tiles.append(g)
        n -= g
    return tiles


def _build_nc(ncl, ns):
    from contextlib import ExitStack

    import concourse.tile as tile
    from concourse import bacc, mybir
    from concourse.masks import make_identity

    f16, f32, f8 = mybir.dt.float16, mybir.dt.float32, mybir.dt.float8e4

    plan8 = _plan_long(ncl)
    plan16 = _plan_chunks(ns, G16)
    nt8, nt16 = len(plan8), len(plan16)

    nc = bacc.Bacc(None, target_bir_lowering=False)
    el_d = nc.declare_dram_parameter("el", [nt8 * BS, G8, CW8], f8,
                                     isOutput=False)
    es_d = None
    if ns:
        es_d = nc.declare_dram_parameter("es", [nt16 * BS, G16, CW16], f16,
                                         isOutput=False)
    aux_d = nc.declare_dram_parameter("aux", [BS, AUXW], f16, isOutput=False)
    out_d = nc.declare_dram_parameter("out", [O, BS], f32, isOutput=True)

    with tile.TileContext(nc) as tc, ExitStack() as ctx:
        sb = ctx.enter_context(tc.tile_pool(name="sb", bufs=1))
        st = ctx.enter_context(tc.tile_pool(name="st", bufs=NBUF))
        ps = ctx.enter_context(tc.tile_pool(name="ps", bufs=1, space="PSUM"))
        ps2 = ctx.enter_context(tc.tile_pool(name="ps2", bufs=2, space="PSUM"))

        # stream DMAs: first two fp8 tiles, then the fp16 side stream and
        # aux (consumed last / at the tail, but they are small -- get them
        # in flight early), then the rest of the fp8 stream
        ltiles = []
        for t, g in enumerate(plan8):
            et = st.tile([BS, g, CW8], f8, tag="el")
            ltiles.append(et)
        stiles = []
        for t, g in enumerate(plan16):
            et = sb.tile([BS, g, CW16], f16, tag=f"es{t}")
            stiles.append(et)
        aux = sb.tile([BS, AUXW], f16, tag="aux")

        # identity (for the tail transposes) doubles as warm-up fodder --
        # emitted FIRST so gpsimd builds it before its SWDGE DMA triggers,
        # and the fp32 warm-up matmuls keep the PE busy while the first
        # stream tile is in flight, so the HAM clock is at 2.4 GHz when
        # the stream starts instead of ramping mid-stream
        ident = sb.tile([128, 128], f32, tag="ident")
        make_identity(nc, ident[:])
        hp = ps.tile([BS, E], f32, tag="hp", space="PSUM")
        # warm-up results land in hp and are discarded by the first real
        # chunk's start=True reset
        for _ in range(NWARM):
            nc.tensor.matmul(out=hp[:, 0:128], lhsT=ident[:], rhs=ident[:],
                             start=True, stop=True)

        # stream DMAs alternate between the two HWDGE rings (sync+scalar)
        # so one ring's fixed per-DMA latency hides under the other ring's
        # transfer; the small side tensors ride the vector engine's queue
        def dma_ltile(t):
            g = plan8[t]
            eng = nc.sync if t % 2 == 0 else nc.scalar
            eng.dma_start(out=ltiles[t][:],
                          in_=el_d[t * BS:(t + 1) * BS, 0:g, :])

        dma_ltile(0)
        if nt8 > 1:
            dma_ltile(1)
        for t, g in enumerate(plan16):
            nc.gpsimd.dma_start(out=stiles[t][:],
                                in_=es_d[t * BS:(t + 1) * BS, 0:g, :])
        nc.gpsimd.dma_start(out=aux[:], in_=aux_d[:])
        for t in range(2, nt8):
            dma_ltile(t)

        # identity (for the tail transposes) doubles as warm-up fodder:
        # fp32 matmuls on it keep the PE busy while the first stream tile
        # is still in flight, so the HAM clock is at 2.4 GHz when the
        # stream starts instead of ramping mid-stream
        # pooled*len*SCALE accumulates over all chunks in one PSUM bank:
        # fp8 long chunks first (DoubleRow: a chunk pair per instruction),
        # fp16 short chunks as the tail
        nct = ncl + ns
        kk = 0
        for t, g in enumerate(plan8):
            et = ltiles[t]
            for k in range(0, g, 2):
                nc.tensor.matmul(
                    out=hp[:],
                    lhsT=et[:, k:k + 2, 0:128],
                    rhs=et[:, k:k + 2, 128:128 + E],
                    start=(kk == 0),
                    stop=(kk + 2 == nct),
                    perf_mode=mybir.MatmulPerfMode.DoubleRow,
                )
                kk += 2
        for t, g in enumerate(plan16):
            et = stiles[t]
            for k in range(g):
                nc.tensor.matmul(
                    out=hp[:],
                    lhsT=et[:, k, 0:128],
                    rhs=et[:, k, 128:128 + E],
                    start=(kk == 0),
                    stop=(kk == nct - 1),
                )
                kk += 1

        # pooled = hp / (len*SCALE)  (f32, then transposed+cast to f16)
        lenf = sb.tile([BS, 1], f32, tag="lenf")
        nc.vector.tensor_copy(out=lenf[:], in_=aux[:, AUX_LEN:AUX_LEN + 1])
        recip = sb.tile([BS, 1], f32, tag="recip")
        nc.vector.reciprocal(recip[:], lenf[:])
        pooled = sb.tile([BS, E], f32, tag="pooled")

        # pooled^T chunks (f16), chunk 2 padded with a ones row (fc1 bias);
        # divide per chunk so each transpose starts as soon as its slice is
        # ready instead of waiting on one full-width division
        pT = []
        for c, (c0, c1) in enumerate([(0, 128), (128, 256), (256, E)]):
            w = c1 - c0
            nc.vector.tensor_scalar(
                out=pooled[:, c0:c1], in0=hp[:, c0:c1],
                scalar1=recip[:, 0:1], scalar2=None,
                op0=mybir.AluOpType.mult,
            )
            pt = ps2.tile([w, 128], f32, tag="tr", space="PSUM")
            nc.tensor.transpose(out=pt[:], in_=pooled[:, c0:c1], identity=ident[:])
            rows = w + 1 if c == 2 else w
            lt = sb.tile([rows, 128], f16, tag=f"pT{c}")
            if c == 2:
                nc.vector.memset(lt[:], 1.0)
            nc.vector.tensor_copy(out=lt[0:w, :], in_=pt[:])
            pT.append(lt)

        # fc1 transposed: hT_j = W1b[:, j]^T @ pooled^T -> relu -> f16
        # (relu split across the scalar and vector engines)
        crows = [(0, 128), (128, 256), (256, E + 1)]
        hT = []
        for j in range(4):
            htp = ps2.tile([128, BS], f32, tag="htp", space="PSUM")
            for c, (r0, r1) in enumerate(crows):
                nc.tensor.matmul(
                    out=htp[:],
                    lhsT=aux[0:r1 - r0, c * 512 + j * 128:c * 512 + (j + 1) * 128],
                    rhs=pT[c][:], start=(c == 0), stop=(c == 2),
                )
            ht = sb.tile([128, BS], f16, tag=f"hT{j}")
            if j % 2:
                nc.scalar.activation(out=ht[:], in_=htp[:],
                                     func=mybir.ActivationFunctionType.Relu)
            else:
                nc.vector.tensor_scalar(
                    out=ht[:], in0=htp[:], scalar1=0.0, scalar2=None,
                    op0=mybir.AluOpType.max,
                )
            hT.append(ht)

        # fc2: out = h @ W2 + b2 (hT_j is already the lhsT layout)
        ones1 = sb.tile([1, BS], f16, tag="ones1")
        nc.vector.memset(ones1[:], 1.0)
        op_ = ps.tile([BS, O], f32, tag="op", space="PSUM")
        for j in range(4):
            nc.tensor.matmul(out=op_[:], lhsT=hT[j][:],
                             rhs=aux[:, AUX_W2[j]:AUX_W2[j] + O],
                             start=(j == 0), stop=False)
        nc.tensor.matmul(out=op_[:], lhsT=ones1[:],
                         rhs=aux[0:1, AUX_B2:AUX_B2 + O],
                         start=False, stop=True)
        # transpose the [128, 2] result to [2, 128] so the store is 2 records
        out_sb = sb.tile([BS, O], f32, tag="osb")
        nc.vector.tensor_copy(out=out_sb[:], in_=op_[:])
        otp = ps2.tile([O, BS], f32, tag="otp", space="PSUM")
        nc.tensor.transpose(out=otp[:], in_=out_sb[:], identity=ident[:])
        oT = sb.tile([O, BS], f32, tag="oT")
        nc.vector.tensor_copy(out=oT[:], in_=otp[:])
        nc.sync.dma_start(out=out_d[:], in_=oT[:])

    nc.finalize()
    return nc


def _pack_stream(cnt, embq, plan, gmax, cw, dtype):
    """Interleave [count row | emb row] per chunk into DMA-tile-major
    layout: out[t*BS + p, k*cw : (k+1)*cw] = chunk (c0+k), row p."""
    ntiles = len(plan)
    out = np.zeros((ntiles * BS, gmax * cw), dtype)
    c0 = 0
    for t, g in enumerate(plan):
        cb = cnt[c0 * 128:(c0 + g) * 128].reshape(g, 128, BS).transpose(1, 0, 2)
        eb = embq[c0 * 128:(c0 + g) * 128].reshape(g, 128, EP).transpose(1, 0, 2)
        blk = np.concatenate([cb, eb], axis=2)  # [128, g, cw]
        out[t * BS:(t + 1) * BS, 0:g * cw] = blk.reshape(BS, g * cw)
        c0 += g
    return out.reshape(ntiles * BS, gmax, cw)


def _prep_in_maps(text, lengths, emb_table, W1, b1, W2, b2):
    text = np.asarray(text).astype(np.int64)        # [S, B]
    lengths = np.asarray(lengths).astype(np.int64)  # [B]
    emb = np.asarray(emb_table, np.float32)
    w1b = np.vstack([np.asarray(W1, np.float32),
                     np.asarray(b1, np.float32)[None, :]]).astype(np.float16)
    w2 = np.asarray(W2, np.float32).astype(np.float16)
    b2f = np.asarray(b2, np.float32).astype(np.float16)

    svec = np.arange(S)[:, None]
    col_tokens = [np.unique(text[:lengths[b], b]) for b in range(B)]
    is_short = lengths <= TSHORT

    # assign columns to cores greedily to minimize the max stream bytes
    # per core (the slowest core's stream sets every core's length);
    # kernel output is unpermuted on the host afterwards
    order = np.argsort([-len(t) for t in col_tokens])
    seen_l = np.zeros((NCORES, V), bool)
    seen_s = np.zeros((NCORES, V), bool)
    counts_n = [0] * NCORES
    dist = [0] * NCORES
    assign = [[] for _ in range(NCORES)]
    for b in order:
        toks = col_tokens[b]
        wb = CW16 * 2 if is_short[b] else CW8
        seen = seen_s if is_short[b] else seen_l
        best, bkey = None, None
        for i in range(NCORES):
            if counts_n[i] >= BS:
                continue
            inc = int(np.count_nonzero(~seen[i, toks])) * wb
            key = (dist[i] + inc, counts_n[i])
            if best is None or key < bkey:
                best, binc, bkey = i, inc, key
        assign[best].append(b)
        counts_n[best] += 1
        dist[best] += binc
        (seen_s if is_short[b] else seen_l)[best, col_tokens[b]] = True
    col_perm = np.concatenate([np.sort(np.array(a, np.int64))
                               for a in assign])

    colid = np.broadcast_to(np.arange(BS)[None, :], (S, BS))
    per_core = []
    nl_max = ns_max = 0
    for i in range(NCORES):
        cols = col_perm[i * BS:(i + 1) * BS]
        t_sh = text[:, cols]
        l_sh = lengths[cols]
        sh_cols = is_short[cols]
        mask = svec < l_sh[None, :]
        used, inv = np.unique(t_sh[mask], return_inverse=True)
        cnt = np.zeros((len(used), BS), np.float32)
        np.add.at(cnt, (inv, colid[mask]), 1.0)
        # tokens needing the exact fp16 path: in a short column, or with a
        # count e3m4 cannot represent exactly (> 15)
        in_short = np.zeros(len(used), bool)
        if sh_cols.any():
            in_short = cnt[:, sh_cols].max(axis=1) > 0
        in_short |= cnt.max(axis=1) > 15
        n_s = int(in_short.sum())
        n_l = len(used) - n_s
        nl_max = max(nl_max, n_l)
        ns_max = max(ns_max, n_s)
        per_core.append((used, cnt, in_short, l_sh))

    ncl = -(-nl_max // 128)
    ncl += ncl % 2  # DoubleRow consumes chunk pairs
    ns = -(-ns_max // 128) if ns_max else 0
    plan8 = _plan_long(ncl)
    plan16 = _plan_chunks(ns, G16)
    nt8, nt16 = len(plan8), len(plan16)

    in_maps = []
    for used, cnt, in_short, l_sh in per_core:
        lt, st_ = used[~in_short], used[in_short]
        # fp8 long stream: counts + scaled e3m4 rows, interleaved
        c8 = np.zeros((ncl * 128, BS), F8)
        c8[:len(lt)] = cnt[~in_short]
        e8 = np.zeros((ncl * 128, EP), F8)
        e8[:len(lt), :E] = (emb[lt] * SCALE).astype(F8)
        el_w = _pack_stream(c8, e8, plan8, G8, CW8, F8)
        # fp16 short stream
        if ns:
            c16 = np.zeros((ns * 128, BS), np.float16)
            c16[:len(st_)] = cnt[in_short]
            e16 = np.zeros((ns * 128, EP), np.float16)
            e16[:len(st_), :E] = (emb[st_] * SCALE).astype(np.float16)
            es_w = _pack_stream(c16, e16, plan16, G16, CW16, np.float16)
        aux = np.zeros((BS, AUXW), np.float16)
        for c, (r0, r1) in enumerate([(0, 128), (128, 256), (256, E + 1)]):
            aux[0:r1 - r0, c * 512:(c + 1) * 512] = w1b[r0:r1]
        for j in range(4):
            aux[:, AUX_W2[j]:AUX_W2[j] + O] = w2[j * 128:(j + 1) * 128]
        aux[0, AUX_B2:AUX_B2 + O] = b2f
        aux[:, AUX_LEN] = (l_sh * SCALE).astype(np.float16)  # exact: 8*len
        m = {"el": np.ascontiguousarray(el_w), "aux": aux}
        if ns:
            m["es"] = np.ascontiguousarray(es_w)
        in_maps.append(m)
    return in_maps, ncl, ns, col_perm


def _run(inputs, trace=False):
    from concourse.bass_utils import run_bass_kernel_spmd

    in_maps, ncl, ns, col_perm = _prep_in_maps(**inputs)
    nc = _build_nc(ncl, ns)
    res = run_bass_kernel_spmd(nc, in_maps, list(range(NCORES)), trace=trace)
    perm_out = np.concatenate(
        [res.results[i]["out"].T for i in range(NCORES)], axis=0)
    out = np.empty_like(perm_out)
    out[col_perm] = perm_out
    return out.astype(np.float32), res


def kernel(**inputs):
    out, _ = _run(inputs, trace=False)
    return out
